# revision 47
# baseline (speedup 1.0000x reference)
"""Trainium2 Bass kernel for the NTN problem.

out[b,k,q,a] = sigmoid( q[b,q,:] @ w[k] @ da[b,a,:]
                        + Vq[k]@q[b,q,:] + Vd[k]@da[b,a,:] + b[k] )

B=64, K=16, Q=A=D=256.  Sharding: data-parallel over batch B across the
8 NeuronCores (8 batches per core); w/V/b replicated.

Per core, per (k, batch-pair):
  MM1 (TensorE, fp16): tmp[e, q|q'] = sum_d w[k,d,e]^T qT[d, q|q']   (N=512)
  DVE: tmp PSUM->SBUF (fp16) with per-partition bias +Vd[k,e] (folds Vd@da)
  MM2 (TensorE, fp16): out[q, a] = sum_e tmp[e,q]^T daT[e, a]
  ScalarE: sigmoid(psum + bias mq[b,k,q]) where mq = Vq@q + b (host-prepped),
  written as fp16 into per-(b, qtile, k-chunk) collect tiles; the host
  upcasts the returned fp16 output to fp32 (quantization err ~2.4e-4,
  well inside the 2e-2 budget) which halves store traffic 32->16 MB/core.

Startup path (from NTFF traces): engine preambles run until ~5us, first
DMA descriptors flow ~8.7us, first loads land ~11us.  The critical first
loads are split across the two HWDGE queues (scalar: q2 pair0 then vdt;
SP: w[0] first, then da2 pair0, w1, mq, w2..w15 in consumer-deadline
order).  PE warm-up matmuls on a raw uninitialized SBUF scratch run from
the moment the global start barrier clears (~6.7us) so the HAM clock
(full speed ~5.4us after continuous-busy start) is ramped when the real
stream begins; warm-up count is sized to end right at load-ready.
Output chunks ship as one dma_start per k-chunk ([128, h, qt, cs, A]
collect tiles, [b,q,k,a] DRAM layout so (k,a) merge into 2KB runs), one
chunk per 4 k's on SP, with a tapered 2-queue flush at the very end.
"""

import os
import sys
import types
from contextlib import ExitStack

if "/opt/trn_rl_repo" not in sys.path:
    sys.path.insert(0, "/opt/trn_rl_repo")

import numpy as np

import concourse.bass as bass
import concourse.tile as tile
from concourse import bacc, bass_utils, mybir

F32 = mybir.dt.float32
F16 = mybir.dt.float16
SIG = mybir.ActivationFunctionType.Sigmoid

NCORES = 8
B, Q, A, D, K = 64, 256, 256, 256, 16
E = D
BL = B // NCORES

N_WARMUP = 16


def _install_profshim():
    """Provide antenv.axon_hooks so trace=True works under axon (best-effort)."""
    try:
        if "antenv.axon_hooks" in sys.modules:
            return True
        import antenv

        mod = types.ModuleType("antenv.axon_hooks")
        holder = {}
        mod.set_axon_ntff_profile_hook = lambda h: holder.__setitem__("h", h)
        mod.get_axon_ntff_profile_hook = lambda: holder.get("h")
        sys.modules["antenv.axon_hooks"] = mod
        antenv.axon_hooks = mod
        from trn_agent_boot.trn_boot import _ntff_profile_via_ctypes

        hook = _ntff_profile_via_ctypes("/opt/axon/libaxon_pjrt.so")
        if hook is None:
            return False
        mod.set_axon_ntff_profile_hook(hook)
        return True
    except Exception:
        return False


def _build_ntn(tc: tile.TileContext, ctx: ExitStack, aps: dict):
    nc = tc.nc
    DC, ET, QT = D // 128, E // 128, Q // 128
    qt, dat, w, vdt, mq, out = (aps[n] for n in ("qt", "dat", "w", "vdt", "mq", "out"))

    w_pool = ctx.enter_context(tc.tile_pool(name="w", bufs=1))
    const_pool = ctx.enter_context(tc.tile_pool(name="const", bufs=1))
    q_pool = ctx.enter_context(tc.tile_pool(name="q", bufs=3))
    da_pool = ctx.enter_context(tc.tile_pool(name="da", bufs=3))
    tmp_pool = ctx.enter_context(tc.tile_pool(name="tmp", bufs=4))
    out_pool = ctx.enter_context(tc.tile_pool(name="out", bufs=6))
    ptmp_pool = ctx.enter_context(tc.tile_pool(name="ptmp", bufs=3, space="PSUM"))
    pout_pool = ctx.enter_context(tc.tile_pool(name="pout", bufs=5, space="PSUM"))

    act_tiles = {}

    # PE warm-up: dummy matmuls on a RAW (bass-level, dependency-free) SBUF
    # scratch region keep the PE busy from the end of its sequencer preamble
    # (~3.5us, the earliest of all engines) until the first real loads land
    # (~10.5us), so the HAM clock ramp (~4-5.5us of continuous busy) finishes
    # before the real stream starts.  A pool tile would need a runtime
    # memset first (tile validation rejects read-before-write), which gated
    # warm-up start at ~7us in earlier traces; reading uninitialized SBUF is
    # fine here since the results land in a write-only PSUM tile.  256-row
    # warm matmuls give fine granularity so the warm-up end lands close to
    # load-ready.  pwarm shares ptmp's ring via the explicit "pt" tag: a
    # pool splits bufs across tags, so an untagged pwarm would silently
    # shrink the per-k pt ring from 3 to 2 (observed as every post-boundary
    # MM1 waiting on the previous k's DVE add).
    scratch_h = ctx.enter_context(nc.sbuf_tensor("warm_scratch", [128, 256], F16))
    scratch = scratch_h.ap()
    pwarm = ptmp_pool.tile([128, 2 * Q], F32, name="pwarm", tag="pt")
    for _ in range(N_WARMUP):
        nc.tensor.matmul(pwarm[:, 0:256], lhsT=scratch[:, 0:128], rhs=scratch[:],
                         start=True, stop=True)

    def load_pair(bp, first=False):
        b0, b1 = 2 * bp, 2 * bp + 1
        q2 = q_pool.tile([128, DC, 2 * Q], F16, name=f"q2_{bp}", tag="q2")
        nc.sync.dma_start(q2[:], qt[bp])
        if first:
            return (q2, b0, b1)
        da2 = da_pool.tile([128, ET, 2 * A], F16, name=f"da2_{bp}", tag="da2")
        nc.sync.dma_start(da2[:], dat[bp])
        act_tiles[bp] = (q2, da2)

    def load_wk(k):
        wk = w_pool.tile([128, DC, E], F16, name=f"wk{k}", tag=f"wk{k}")
        nc.sync.dma_start(wk[:], w[k].rearrange("(dc p) e -> p dc e", p=128))
        return wk

    # Critical path: MM1(k0) needs q2_0 + wk0.  Put q2_0 (256KB) alone on the
    # scalar HWDGE queue and wk0 (128KB) first on the SP queue so both stream
    # concurrently the moment descriptors start flowing; the rest of SP's
    # queue follows consumer deadlines: MM1(k1)->MM2(k0)->DVE(k0)->ACT(k0).
    q2_0 = q_pool.tile([128, DC, 2 * Q], F16, name="q2_0", tag="q2")
    nc.scalar.dma_start(q2_0[:], qt[0])
    # SP-queue order wk0, da2_0, w1 follows consumer deadlines (MM1 k0,
    # MM2 k0, MM1 k1).  Alternatives measured the same within run-to-run
    # DMA jitter (w1-first 132.9, per-h da2 split 133.1, this 132.4): all
    # three early deadlines sit ~1-2us after stream start behind ~512KB of
    # queue traffic, and which one slips ~1us varies by run.
    w_sb = {}
    w_sb[0] = load_wk(0)
    da2_0 = da_pool.tile([128, ET, 2 * A], F16, name="da2_0", tag="da2")
    nc.sync.dma_start(da2_0[:], dat[0])
    act_tiles[0] = (q2_0, da2_0)
    w_sb[1] = load_wk(1)
    # vdt (DVE k0's bias) and mq (first sigmoid's bias) ride the otherwise
    # idle scalar queue behind q2_0 — both have >=1us of deadline slack
    # there, and keeping mq off SP advances w2..w15 by ~0.7us (mq ahead of
    # w2 on SP was seen slipping MM1(k2) by ~2us on jittery runs).
    vdt_sb = const_pool.tile([128, ET, 128], F32)
    nc.scalar.dma_start(vdt_sb[:], vdt.rearrange("et p k -> p et k"))
    mq_sb = const_pool.tile([128, QT, BL, K], F32)
    nc.scalar.dma_start(mq_sb[:], mq.rearrange("t p b k -> p t b k"))
    # w2..w7 stay one-dma_start-per-k: bulk-merging from w3 starved the
    # early back-to-back MM1 cadence (measured +2.9us PE gap).  w8..w15 are
    # needed >=20us in (~4us margin even at the early MM1 cadence), so they
    # ride one bulk dma_start — 7 fewer queue entries trims the per-entry
    # end-of-kernel drain and sequencer configs.
    for k in range(2, 8):
        w_sb[k] = load_wk(k)
    w_bulk = w_pool.tile([128, 8, DC, E], F16, name="w_bulk", tag="w_bulk")
    nc.sync.dma_start(w_bulk[:], w[8:K].rearrange("k (dc p) e -> p k dc e", p=128))
    for k in range(8, K):
        w_sb[k] = w_bulk[:, k - 8]

    NBP = BL // 2
    # One collect tile per k-chunk covers both batches and both q-tiles
    # ([128, h, qt, cs, A]), so a chunk ships as a SINGLE dma_start (fewer
    # ~0.8-1.3us DGE configs serializing on the sequencers, and a shorter
    # end-of-kernel queue drain).  Chunks complete at their boundary k and
    # the store issues right there on the SP queue; the last two tapered
    # chunks split h across SP/Scalar so the final flush runs two configs
    # in parallel.
    for bp in range(NBP):
        b0, b1 = 2 * bp, 2 * bp + 1
        if bp not in act_tiles:
            load_pair(bp)
        if bp + 1 < NBP and bp + 1 not in act_tiles:
            load_pair(bp + 1)
        q2, da2 = act_tiles.pop(bp)

        last_bp = bp == NBP - 1
        if not last_bp:
            chunk_sizes = [8, 8]
        else:
            chunk_sizes = [4, 4, 4, 2, 1, 1]
        k2chunk = {}
        koff = 0
        for ci, cs in enumerate(chunk_sizes):
            for off in range(cs):
                k2chunk[koff + off] = (ci, off, cs)
            koff += cs
        coll = {ci: out_pool.tile([128, 2, QT, cs, A], F16, name="coll", tag="coll")
                for ci, cs in enumerate(chunk_sizes)}

        for k in range(K):
            ptmps = [ptmp_pool.tile([128, 2 * Q], F32, name=f"pt{et}", tag="pt")
                     for et in range(ET)]
            for et in range(ET):
                for dc in range(DC):
                    nc.tensor.matmul(
                        ptmps[et][:],
                        lhsT=w_sb[k][:, dc, et * 128:(et + 1) * 128],
                        rhs=q2[:, dc, :],
                        start=(dc == 0),
                        stop=(dc == DC - 1),
                    )
            tmp = tmp_pool.tile([128, ET, 2 * Q], F16)
            if last_bp and k == K - 1:
                # final k: per-h half-width adds so MM2(h0) unblocks ~0.35us
                # sooner, pulling in the end-of-kernel sigmoid/store chain
                for h in (0, 1):
                    for et in range(ET):
                        nc.vector.tensor_scalar_add(
                            tmp[:, et, h * Q:(h + 1) * Q],
                            ptmps[et][:, h * Q:(h + 1) * Q],
                            vdt_sb[:, et, k:k + 1],
                        )
            else:
                for et in range(ET):
                    nc.vector.tensor_scalar_add(
                        tmp[:, et, :], ptmps[et][:], vdt_sb[:, et, k:k + 1]
                    )
            for h, b in ((0, b0), (1, b1)):
                for qt_i in range(QT):
                    po = pout_pool.tile([128, A], F32)
                    for et in range(ET):
                        nc.tensor.matmul(
                            po[:],
                            lhsT=tmp[:, et, h * Q + qt_i * 128: h * Q + (qt_i + 1) * 128],
                            rhs=da2[:, et, h * A:(h + 1) * A],
                            start=(et == 0),
                            stop=(et == ET - 1),
                        )
                    nc.scalar.activation(
                        coll[k2chunk[k][0]][:, h, qt_i, k2chunk[k][1], :], po[:], SIG,
                        bias=mq_sb[:, qt_i, b, k:k + 1],
                    )
            ci, off, cs = k2chunk[k]
            if off == cs - 1:
                k_lo = k - cs + 1
                # out DRAM layout is [b, q, k, a] so (k, a) merges into one
                # contiguous 512*cs-byte run per (b, q) — 3 free dims (DMA AP
                # limit) and fat descriptors; host restores [b, k, q, a].
                dram = out[b0:b0 + 2, :, k_lo:k_lo + cs].rearrange(
                    "h (qt p) k a -> p h qt k a", p=128)
                if last_bp and ci >= len(chunk_sizes) - 2:
                    # final taper chunks: h=0 on SP, h=1 on Scalar, in parallel
                    nc.sync.dma_start(dram[:, 0], coll[ci][:, 0])
                    nc.scalar.dma_start(dram[:, 1], coll[ci][:, 1])
                else:
                    nc.sync.dma_start(dram, coll[ci][:])


_COMPILED = None


def _get_compiled():
    global _COMPILED
    if _COMPILED is not None:
        return _COMPILED
    nc = bacc.Bacc("TRN2", target_bir_lowering=False, debug=False, num_devices=NCORES)
    aps = {
        "qt": nc.dram_tensor("qt", [BL // 2, 128, D // 128, 2 * Q], F16,
                             kind="ExternalInput").ap(),
        "dat": nc.dram_tensor("dat", [BL // 2, 128, E // 128, 2 * A], F16,
                              kind="ExternalInput").ap(),
        "w": nc.dram_tensor("w", [K, D, E], F16, kind="ExternalInput").ap(),
        "vdt": nc.dram_tensor("vdt", [E // 128, 128, 128], F32, kind="ExternalInput").ap(),
        "mq": nc.dram_tensor("mq", [Q // 128, 128, BL, K], F32, kind="ExternalInput").ap(),
        "out": nc.dram_tensor("out", [BL, Q, K, A], F16, kind="ExternalOutput").ap(),
    }
    with tile.TileContext(nc) as tc:
        with ExitStack() as ctx:
            _build_ntn(tc, ctx, aps)
    nc.compile()
    _COMPILED = nc
    return nc


def kernel(batch_q_em, batch_da_em, w, V, b):
    q = np.ascontiguousarray(np.asarray(batch_q_em, dtype=np.float32))
    da = np.ascontiguousarray(np.asarray(batch_da_em, dtype=np.float32))
    w = np.ascontiguousarray(np.asarray(w, dtype=np.float32))
    V = np.ascontiguousarray(np.asarray(V, dtype=np.float32))
    b = np.asarray(b, dtype=np.float32).reshape(-1)

    # packed per batch-PAIR to the exact SBUF tile image [bp, p, dc, (h q)]
    # so each load is DC contiguous 2KB runs per partition (fat descriptors)
    qt = np.ascontiguousarray(
        q.transpose(0, 2, 1).reshape(B, D // 128, 128, Q).transpose(0, 2, 1, 3)
        .reshape(B // 2, 2, 128, D // 128, Q).transpose(0, 2, 3, 1, 4)
        .reshape(B // 2, 128, D // 128, 2 * Q)
    ).astype(np.float16)                                  # [B/2, 128, DC, 2Q]
    dat = np.ascontiguousarray(
        da.transpose(0, 2, 1).reshape(B, E // 128, 128, A).transpose(0, 2, 1, 3)
        .reshape(B // 2, 2, 128, E // 128, A).transpose(0, 2, 3, 1, 4)
        .reshape(B // 2, 128, E // 128, 2 * A)
    ).astype(np.float16)                                  # [B/2, 128, ET, 2A]
    w16 = w.astype(np.float16)
    vdt_cols = np.ascontiguousarray(V[:, D:].T)          # [E, K]
    vdt = np.zeros((E // 128, 128, 128), dtype=np.float32)
    vdt[:, :, :K] = vdt_cols.reshape(E // 128, 128, K)
    # mq[b,q,k] = q[b] @ Vq^T + bias
    mqT = q @ V[:, :D].T + b[None, None, :]              # [B, Q, K]

    nc = _get_compiled()
    in_maps = []
    for c in range(NCORES):
        s = slice(c * BL, (c + 1) * BL)
        sp = slice(c * (BL // 2), (c + 1) * (BL // 2))
        mq_shard = np.ascontiguousarray(
            mqT[s].reshape(BL, Q // 128, 128, K).transpose(1, 2, 0, 3)
        )  # [QT, 128, BL, K]
        in_maps.append({
            "qt": np.ascontiguousarray(qt[sp]),
            "dat": np.ascontiguousarray(dat[sp]),
            "w": w16,
            "vdt": vdt,
            "mq": mq_shard,
        })

    trace = bool(int(os.environ.get("NTN_TRACE", "0"))) and _install_profshim()
    res = bass_utils.run_bass_kernel_spmd(
        nc, in_maps, core_ids=list(range(NCORES)), trace=trace
    )
    if trace and res.exec_time_ns is not None:
        print(f"HW exec time: {res.exec_time_ns} ns")
    out = np.concatenate([r["out"] for r in res.results], axis=0)  # [B, Q, K, A] f16
    return np.ascontiguousarray(out.transpose(0, 2, 1, 3), dtype=np.float32)
